# revision 10
# baseline (speedup 1.0000x reference)
"""Correlation kernel (FlowNet-style, W-displacement only) for Trainium2.

out[b, j, h, w] = mean_c f1[b,c,h,w] * f2pad[b,c,h,w+j],  j in [0, 81), pad=40.

Sharding: data-parallel over batch B=8 across 8 cores (1 batch elem/core).

Per-core pipeline (per h row):
  1. 3 matmuls (contraction over C=128 on partitions) produce Gram tiles
     G^T[w, u] = sum_c f1[c, w0+w] * f2p[c, w0+u] in PSUM.
  2. DVE/ACT copy PSUM -> SBUF.
  3. Band extraction: SBUF diagonal APs are illegal (partition steps must be
     partition-aligned), so bounce through DRAM: dump G^T tiles densely to a
     DRAM scratch, read back with a diagonal DRAM-side AP (flat, legal) so
     partition p holds out[p-th w, j=0..80].
  4. 3 PE transposes (identity matmul) -> PSUM tile [81, 320] (j on partitions).
  5. ACT copy (x 1/128) -> SBUF staging; chunk-batched contiguous DMA to DRAM.
"""

import numpy as np
from contextlib import ExitStack

B, C, H, W = 8, 128, 96, 320
D = 40
J = 2 * D + 1  # 81
WP = W + 2 * D  # 400
N_CORES = 8

HCHUNK = 16
NCHUNK = H // HCHUNK
# w-block starts; all matmuls padded to uniform M=128 (last block reads 64
# slack columns of garbage that the transpose never consumes)
WB = [0, 128, 256]
GN = 208  # matmul free dim / per-block width in gsb (= 128 + 2*D)
SLACK = 64


def _build(h_total=H):
    import concourse.bass as bass
    import concourse.tile as tile
    from concourse import bacc, mybir
    from concourse.masks import make_identity

    dt = mybir.dt.float32
    nc = bacc.Bacc(
        "TRN2",
        target_bir_lowering=False,
        debug=False,
        enable_asserts=False,
        num_devices=N_CORES,
    )
    f1 = nc.dram_tensor("f1", [C, h_total, W], dt, kind="ExternalInput").ap()
    f2 = nc.dram_tensor("f2", [C, h_total, W], dt, kind="ExternalInput").ap()
    out = nc.dram_tensor("out", [J, h_total, W], dt, kind="ExternalOutput").ap()

    nchunk = h_total // HCHUNK

    with tile.TileContext(nc) as tc, ExitStack() as ctx:
        const_pool = ctx.enter_context(tc.tile_pool(name="const", bufs=1))
        scr_pool = ctx.enter_context(tc.tile_pool(name="scr", bufs=4, space="DRAM"))
        f1_pool = ctx.enter_context(tc.tile_pool(name="f1p", bufs=2))
        f2_pool = ctx.enter_context(tc.tile_pool(name="f2p", bufs=2))
        g_pool = ctx.enter_context(tc.tile_pool(name="gsb", bufs=3))
        ral_pool = ctx.enter_context(tc.tile_pool(name="ral", bufs=3))
        ost_pool = ctx.enter_context(tc.tile_pool(name="ost", bufs=2))
        psg_pool = ctx.enter_context(tc.tile_pool(name="psg", bufs=6, space="PSUM"))
        pst_pool = ctx.enter_context(tc.tile_pool(name="pst", bufs=2, space="PSUM"))

        ident = const_pool.tile([128, 128], dt)
        make_identity(nc, ident[:])

        for ci in range(nchunk):
            h0 = ci * HCHUNK
            f1s = f1_pool.tile([C, HCHUNK * W + SLACK], dt)
            nc.gpsimd.memset(f1s[:, HCHUNK * W :], 0.0)
            nc.sync.dma_start(f1s[:, 0 : HCHUNK * W], f1[:, h0 : h0 + HCHUNK, :])
            f2ps = f2_pool.tile([C, HCHUNK * WP + SLACK], dt)
            f2v = f2ps[:, 0 : HCHUNK * WP].rearrange("p (h w) -> p h w", h=HCHUNK)
            # zero the pad columns + slack, then land the data between them
            nc.gpsimd.memset(f2v[:, :, 0:D], 0.0)
            nc.gpsimd.memset(f2v[:, :, W + D : WP], 0.0)
            nc.gpsimd.memset(f2ps[:, HCHUNK * WP :], 0.0)
            nc.sync.dma_start(f2v[:, :, D : W + D], f2[:, h0 : h0 + HCHUNK, :])

            ost = ost_pool.tile([J, HCHUNK * W], dt)
            for h in range(HCHUNK):
                base1 = h * W
                base2 = h * WP
                gsb = g_pool.tile([C, 3 * GN], dt)
                for bi, w0 in enumerate(WB):
                    pg = psg_pool.tile([128, GN], dt, tag="pg")
                    nc.tensor.matmul(
                        pg[:],
                        lhsT=f1s[:, base1 + w0 : base1 + w0 + 128],
                        rhs=f2ps[:, base2 + w0 : base2 + w0 + GN],
                        start=True,
                        stop=True,
                    )
                    if bi < 2:
                        nc.vector.tensor_copy(gsb[:, bi * GN : (bi + 1) * GN], pg[:])
                    else:
                        nc.scalar.copy(gsb[:, bi * GN : (bi + 1) * GN], pg[:])

                # band extraction via DRAM bounce: dense dump, diagonal read-back
                scr = scr_pool.tile([C, 3 * GN], dt)
                nc.sync.dma_start(scr[:], gsb[:])
                ss = scr[:]
                diag_src = bass.AP(
                    ss.tensor, ss.offset, [[ss.ap[0][0] + 1, 128], [GN, 3], [1, J]]
                )
                ral = ral_pool.tile([C, 3 * J], dt)
                rs = ral[:]
                diag_dst = bass.AP(
                    rs.tensor, rs.offset, [[rs.ap[0][0], 128], [J, 3], [1, J]]
                )
                nc.sync.dma_start(diag_dst, diag_src)

                pt = pst_pool.tile([J, W], dt, tag="pt")
                for bi, w0 in enumerate(WB):
                    kp = min(128, W - w0)
                    nc.tensor.transpose(
                        pt[0:J, w0 : w0 + kp],
                        ral[0:kp, bi * J : bi * J + J],
                        ident[0:kp, 0:kp],
                    )
                nc.scalar.mul(ost[:, base1 : base1 + W], pt[:], 1.0 / C)

            nc.sync.dma_start(out[:, h0 : h0 + HCHUNK, :], ost[:])

    nc.finalize()
    return nc


def _run(nc, in_maps, **kwargs):
    from concourse.bass_utils import run_bass_kernel_spmd

    return run_bass_kernel_spmd(nc, in_maps, core_ids=list(range(N_CORES)), **kwargs)


def kernel(f1: np.ndarray, f2: np.ndarray, **run_kwargs) -> np.ndarray:
    assert f1.shape == (B, C, H, W) and f2.shape == (B, C, H, W)
    nc = _build()
    in_maps = [
        {
            "f1": np.ascontiguousarray(f1[i], dtype=np.float32),
            "f2": np.ascontiguousarray(f2[i], dtype=np.float32),
        }
        for i in range(N_CORES)
    ]
    res = _run(nc, in_maps, **run_kwargs)
    out = np.stack([r["out"] for r in res.results], axis=0)
    if run_kwargs:
        kernel.last_results = res
    return out
